# revision 17
# baseline (speedup 1.0000x reference)
"""Trainium2 Bass kernel for the BronxLayer GNN message-passing block.

Computes, for N=8192, IN_F=256, OUT_F=128, EMB=16:
    hh  = eps + h @ W_fc
    k   = hh @ W_k ;  q = hh @ W_q
    a   = a_str * softmax(k @ q.T, axis=-1)
    a   = a / a.sum(-1, keepdims=True)
    out = a @ hh

Sharding: rows of the N dimension are split across 8 NeuronCores (1024
rows each).  Each core receives its a_str row-slice pre-transposed
(a_str[rows].T, bf16) so the kernel works in a [j, i] layout and never
transposes the big matrix on chip, plus replicated h.T / eps.T (fp16)
and small per-core h[rows].T / eps[rows].T used to build this core's
k rows.

Precision plan (validated against the fp32 reference, ~0.5% scale-
relative max error):
  * h/eps/W path is fp16 (scores need ~1e-2 absolute accuracy; fp16
    keeps them there while making every phase-0 matmul single-pass --
    fp32 matmuls cost 2 PE passes on TRN2).
  * scores matmul fp16, accumulated in fp32 PSUM; exp runs on fp32.
  * the a = a_str * exp(scores) tensor and hh_aug are bf16: the final
    matmul is a renormalized weighted average, so per-element rounding
    largely cancels, and bf16 halves the dominant a_str DMA stream.

Algebraic simplifications:
  * softmax's denominator cancels in the renormalization -> skipped.
  * max-subtraction unnecessary: scores ~ N(0, 8^2), exp(|max|~48)
    fits fp32/bf16 range easily.
  * the renorm row-sum falls out of the final matmul by appending a
    ones column to hh (hh_aug, 129 columns per 128-row chunk); one
    per-partition scaled copy performs the division.

PE-array usage:
  * scores have contraction K=EMB=16, so they run 4-way row-packed
    (tile_position=(32r,0)): four j-chunks' score matmuls execute
    concurrently in 32-row strips of the PE array.
  * phase 1 processes groups of (4 j-chunks x 512 i-columns): one
    512KB DMA, 4 packed score matmuls, one [128,2048] exp, one bf16
    multiply, 16 accumulating final matmuls.
"""

import os
import sys

import numpy as np

try:
    import ml_dtypes
    _BF16_NP = ml_dtypes.bfloat16
except ImportError:  # pragma: no cover
    _BF16_NP = None


def _ensure_import_paths():
    for p in (
        "/root/.axon_site",
        "/root/.axon_site/_ro/trn_rl_repo",
        "/root/.axon_site/_ro/pypackages",
        "/opt/trn_rl_repo",
    ):
        if os.path.isdir(p) and p not in sys.path:
            sys.path.append(p)


try:  # noqa: SIM105
    import concourse.bacc  # noqa: F401
except ImportError:
    _ensure_import_paths()

import concourse.bacc as bacc
import concourse.tile as tile
from concourse import mybir
from concourse.bass_utils import run_bass_kernel_spmd

N, IN_F, OUT_F, EMB = 8192, 256, 128, 16
CORES = 8
R = N // CORES            # rows per core (1024)
JC = N // 128             # 64 j-chunks of 128
IC = R // 128             # 8 i-chunks per core
NC_GROUPS = JC // 4       # 16 groups of 4 j-chunks
F32 = mybir.dt.float32
F16 = mybir.dt.float16
BF16 = mybir.dt.bfloat16

_CACHE = {}


def _build():
    """Build + compile the per-core Bass module (identical on all cores)."""
    nc = bacc.Bacc("TRN2", target_bir_lowering=False, debug=False,
                   num_devices=CORES)

    astrT = nc.dram_tensor("astrT", [N, R], BF16, kind="ExternalInput")
    hT = nc.dram_tensor("hT", [IN_F, N], F16, kind="ExternalInput")
    epsT = nc.dram_tensor("epsT", [OUT_F, N], F16, kind="ExternalInput")
    hTc = nc.dram_tensor("hTc", [IN_F, R], F16, kind="ExternalInput")
    epsTc = nc.dram_tensor("epsTc", [OUT_F, R], F16, kind="ExternalInput")
    wfc = nc.dram_tensor("wfc", [IN_F, OUT_F], F16, kind="ExternalInput")
    wk = nc.dram_tensor("wk", [OUT_F, EMB], F16, kind="ExternalInput")
    wq = nc.dram_tensor("wq", [OUT_F, EMB], F16, kind="ExternalInput")
    idn = nc.dram_tensor("idn", [128, 128], F16, kind="ExternalInput")
    outc = nc.dram_tensor("outc", [R, OUT_F], F32, kind="ExternalOutput")

    EXP = mybir.ActivationFunctionType.Exp
    COPY = mybir.ActivationFunctionType.Copy
    MUL = mybir.AluOpType.mult

    with tile.TileContext(nc) as tc:
        with tc.tile_pool(name="persist", bufs=1) as pp:
            # Long-lived SBUF tensors.
            hh_aug = pp.tile([128, JC * 129], BF16, tag="hh_aug")
            qT_pk = pp.tile([128, NC_GROUPS * 128], F16, tag="qT_pk")
            kTc_pk = pp.tile([128, R], F16, tag="kTc_pk")
            wfc0 = pp.tile([128, OUT_F], F16, tag="wfc0")
            wfc1 = pp.tile([128, OUT_F], F16, tag="wfc1")
            wk_sb = pp.tile([OUT_F, EMB], F16, tag="wk")
            wq_sb = pp.tile([OUT_F, EMB], F16, tag="wq")
            idn_sb = pp.tile([128, 128], F16, tag="idn")

            nc.sync.dma_start(out=wfc0[:, :], in_=wfc[0:128, :])
            nc.sync.dma_start(out=wfc1[:, :], in_=wfc[128:256, :])
            nc.sync.dma_start(out=wk_sb[:, :], in_=wk[:, :])
            nc.sync.dma_start(out=wq_sb[:, :], in_=wq[:, :])
            nc.sync.dma_start(out=idn_sb[:, :], in_=idn[:, :])

            # ---------------- Phase 0 ----------------
            with tc.tile_pool(name="ph0", bufs=1) as p0, \
                 tc.tile_pool(name="ph0s", bufs=2) as p0s, \
                 tc.tile_pool(name="ph0ps", bufs=2, space="PSUM") as p0ps, \
                 tc.tile_pool(name="ph0t", bufs=2, space="PSUM") as p0t:
                hhT_sb = p0.tile([OUT_F, N], F16, tag="hhT")
                epsT_sb = p0.tile([OUT_F, N], F16, tag="epsT")
                qT_st = p0.tile([EMB, N], F16, tag="qT_st")
                kTc_sb = p0.tile([EMB, R], F16, tag="kTc_st")
                hT0_sb = p0.tile([128, N], F16, tag="hT0")
                hT1_sb = p0.tile([128, N], F16, tag="hT1")
                ph0_gates = []
                for hs in (slice(0, 2048), slice(2048, 4096),
                           slice(4096, 8192)):
                    d1 = nc.sync.dma_start(out=hT0_sb[:, hs],
                                           in_=hT[0:128, hs])
                    d2 = nc.sync.dma_start(out=hT1_sb[:, hs],
                                           in_=hT[128:256, hs])
                    d3 = nc.gpsimd.dma_start(out=epsT_sb[:, hs],
                                             in_=epsT[:, hs])
                ph0_gates = [d2.ins, d3.ins]

                # This core's k rows first (small, unblocks phase 1):
                # hhTc = W_fc.T @ hTc + epsTc, kTc = W_k.T @ hhTc,
                # replicated to all 4 PE row strips.
                hhTc_sb = p0.tile([OUT_F, R], F16, tag="hhTc")
                htc0 = p0.tile([128, R], F16, tag="htc0")
                htc1 = p0.tile([128, R], F16, tag="htc1")
                epstc = p0.tile([OUT_F, R], F16, tag="epstc")
                nc.gpsimd.dma_start(out=htc0[:, :], in_=hTc[0:128, :])
                nc.gpsimd.dma_start(out=htc1[:, :], in_=hTc[128:256, :])
                nc.gpsimd.dma_start(out=epstc[:, :], in_=epsTc[:, :])
                for f2 in range(2):
                    fs = slice(f2 * 512, (f2 + 1) * 512)
                    psc = p0ps.tile([128, 512], F32, tag="ps_hhT")
                    nc.tensor.matmul(psc[:, :], wfc0[:, :], htc0[:, fs],
                                     start=True, stop=False)
                    nc.tensor.matmul(psc[:, :], wfc1[:, :], htc1[:, fs],
                                     start=False, stop=True)
                    nc.vector.tensor_tensor(hhTc_sb[:, fs], psc[:, :],
                                            epstc[:, fs],
                                            mybir.AluOpType.add)
                for f2 in range(2):
                    fs = slice(f2 * 512, (f2 + 1) * 512)
                    psk = p0ps.tile([EMB, 512], F32, tag="ps_q")
                    nc.tensor.matmul(psk[:, :], wk_sb[:, :], hhTc_sb[:, fs],
                                     start=True, stop=True)
                    nc.vector.tensor_copy(kTc_sb[:, fs], psk[:, :])
                for r in range(4):
                    nc.gpsimd.dma_start(
                        out=kTc_pk[32 * r:32 * r + 16, :], in_=kTc_sb[:, :])
                qst3 = qT_st[:, :].rearrange("p (c blk) -> p c blk", blk=128)

                # hhT = W_fc.T @ hT + epsT, 16 column chunks of 512;
                # qT staging right behind each chunk.
                for f in range(16):
                    fs = slice(f * 512, (f + 1) * 512)
                    ps = p0ps.tile([128, 512], F32, tag="ps_hhT")
                    nc.tensor.matmul(ps[:, :], wfc0[:, :], hT0_sb[:, fs],
                                     start=True, stop=False)
                    nc.tensor.matmul(ps[:, :], wfc1[:, :], hT1_sb[:, fs],
                                     start=False, stop=True)
                    nc.vector.tensor_tensor(hhT_sb[:, fs], ps[:, :],
                                            epsT_sb[:, fs],
                                            mybir.AluOpType.add)
                    psq = p0ps.tile([EMB, 512], F32, tag="ps_q")
                    nc.tensor.matmul(psq[:, :], wq_sb[:, :], hhT_sb[:, fs],
                                     start=True, stop=True)
                    nc.scalar.activation(qT_st[:, fs], psq[:, :],
                                         mybir.ActivationFunctionType.Copy)
                    pst = p0t.tile([128, 512], F16, tag="ps_t", name="pst")
                    for jj in range(4):
                        j = f * 4 + jj
                        js = slice(j * 128, (j + 1) * 128)
                        nc.tensor.matmul(pst[:, jj * 128:(jj + 1) * 128],
                                         hhT_sb[:, js],
                                         idn_sb[:, :], is_transpose=True,
                                         start=True, stop=True)
                    dst = hh_aug[:, :].rearrange(
                        "p (j c) -> p j c", c=129)[:, f * 4:f * 4 + 4, 0:128]
                    src4 = pst[:, :].rearrange("p (j c) -> p j c", c=128)
                    nc.vector.tensor_copy(dst, src4)
                    if f % 4 == 3:
                        q = f // 4
                        for r in range(4):
                            dstq = qT_pk[32 * r:32 * r + 16,
                                         q * 512:(q + 1) * 512].rearrange(
                                "p (c blk) -> p c blk", blk=128)
                            nc.gpsimd.dma_start(
                                out=dstq,
                                in_=qst3[:, 16 * q + r:16 * (q + 1):4, :])

                # hh_aug ones columns.
                nc.vector.memset(hh_aug[:, 128::129], 1.0)


            # ---------------- Phase 1 ----------------
            # Groups of (4 j-chunks x 512 i-columns): 4-way row-packed
            # score matmuls -> exp -> bf16 multiply -> 16 accumulating
            # final matmuls.  8 accumulators of [128,129] pack 3 per
            # PSUM bank; each bank is one accumulation group
            # (per-element has_written handles first-write-overwrites).
            order = [(c, fh, hg, r2, k)
                     for c in range(NC_GROUPS)
                     for fh in range(2)
                     for hg in range(2)
                     for r2 in range(2)
                     for k in range(4)]
            bank_first = {}
            bank_last = {}
            for pos, (c, fh, hg, r2, k) in enumerate(order):
                b = (fh * 4 + k) // 3
                bank_first.setdefault(b, pos)
                bank_last[b] = pos

            with tc.tile_pool(name="acc", bufs=1, space="PSUM") as accp, \
                 tc.tile_pool(name="sps", bufs=2, space="PSUM") as spsp, \
                 tc.tile_pool(name="astr", bufs=8) as astrp, \
                 tc.tile_pool(name="aT", bufs=4) as aTp, \
                 tc.tile_pool(name="outp", bufs=2) as outp:
                accb = [accp.tile([128, 512], F32, tag=f"acc{b}",
                                  name=f"acc{b}")
                        for b in range(3)]

                def acc_ap(i):
                    return accb[i // 3][:, (i % 3) * 129:(i % 3) * 129 + 129]

                pos = 0
                for c in range(NC_GROUPS):
                    for fh in range(2):
                        fhs = slice(fh * 512, (fh + 1) * 512)
                        at = astrp.tile([128, 2048], BF16, tag="astr_t")
                        src = astrT[c * 512:(c + 1) * 512, fhs].rearrange(
                            "(r p) f -> p r f", p=128)
                        dst = at[:, :].rearrange("p (r f) -> p r f", f=512)
                        dd = nc.sync.dma_start(out=dst, in_=src)
                        for g in ph0_gates:
                            tile.add_dep_helper(
                                dd.ins, g, sync=False,
                                reason="astr prefetch after phase-0 inputs")

                        for hg in range(2):
                            pss = spsp.tile([128, 1024], F32, tag="ps_s")
                            for r2 in range(2):
                                r = hg * 2 + r2
                                nc.tensor.matmul(
                                    pss[:, r2 * 512:(r2 + 1) * 512],
                                    qT_pk[32 * r:32 * r + 16,
                                          c * 128:(c + 1) * 128],
                                    kTc_pk[32 * r:32 * r + 16, fhs],
                                    start=True, stop=True,
                                    tile_position=(32 * r, 0))

                            aT_t = aTp.tile([128, 1024], BF16, tag="aT_t")
                            nc.scalar.activation(aT_t[:, :], pss[:, :], EXP)
                            nc.vector.tensor_tensor(
                                aT_t[:, :], aT_t[:, :],
                                at[:, hg * 1024:(hg + 1) * 1024], MUL)

                            for r2 in range(2):
                                r = hg * 2 + r2
                                j = c * 4 + r
                                rhs = hh_aug[:, j * 129:(j + 1) * 129]
                                for k in range(4):
                                    i = fh * 4 + k
                                    b = i // 3
                                    nc.tensor.matmul(
                                        acc_ap(i),
                                        aT_t[:, r2 * 512 + k * 128:
                                             r2 * 512 + (k + 1) * 128],
                                        rhs,
                                        start=(bank_first[b] == pos),
                                        stop=(bank_last[b] == pos),
                                        skip_group_check=True)
                                    pos += 1

                # Epilogue: out rows = numerator / row-sum.
                for i in range(IC):
                    ap = acc_ap(i)
                    rec = outp.tile([128, 1], F32, tag="rec")
                    nc.vector.reciprocal(rec[:, :], ap[:, 128:129])
                    ot = outp.tile([128, OUT_F], F32, tag="ot")
                    nc.vector.tensor_scalar_mul(ot[:, :], ap[:, 0:128],
                                                rec[:, 0:1])
                    eng = nc.scalar if i % 2 == 0 else nc.sync
                    eng.dma_start(
                        out=outc[i * 128:(i + 1) * 128, :], in_=ot[:, :])

    nc.compile()
    return nc


def get_module():
    if "nc" not in _CACHE:
        _CACHE["nc"] = _build()
    return _CACHE["nc"]


def make_in_maps(h, eps, a_str, W_fc, W_k, W_q):
    h = np.asarray(h, np.float32)
    eps = np.asarray(eps, np.float32)
    a_str = np.asarray(a_str, np.float32)
    hT = np.ascontiguousarray(h.T).astype(np.float16)
    epsT = np.ascontiguousarray(eps.T).astype(np.float16)
    idn = np.eye(128, dtype=np.float16)
    common = {
        "hT": hT,
        "epsT": epsT,
        "wfc": np.asarray(W_fc, np.float32).astype(np.float16),
        "wk": np.asarray(W_k, np.float32).astype(np.float16),
        "wq": np.asarray(W_q, np.float32).astype(np.float16),
        "idn": idn,
    }
    in_maps = []
    for c in range(CORES):
        rows = slice(c * R, (c + 1) * R)
        in_maps.append({
            "astrT": np.ascontiguousarray(a_str[rows].T).astype(_BF16_NP),
            "hTc": np.ascontiguousarray(h[rows].T).astype(np.float16),
            "epsTc": np.ascontiguousarray(eps[rows].T).astype(np.float16),
            **common,
        })
    return in_maps


def kernel(h, eps, a_str, W_fc, W_k, W_q):
    nc = get_module()
    in_maps = make_in_maps(h, eps, a_str, W_fc, W_k, W_q)
    res = run_bass_kernel_spmd(nc, in_maps, list(range(CORES)))
    out = np.concatenate([res.results[c]["outc"] for c in range(CORES)],
                         axis=0)
    return out.astype(np.float32)


# revision 18
# speedup vs baseline: 1.2142x; 1.2142x over previous
"""Trainium2 Bass kernel for the BronxLayer GNN message-passing block.

Computes, for N=8192, IN_F=256, OUT_F=128, EMB=16:
    hh  = eps + h @ W_fc
    k   = hh @ W_k ;  q = hh @ W_q
    a   = a_str * softmax(k @ q.T, axis=-1)
    a   = a / a.sum(-1, keepdims=True)
    out = a @ hh

Sharding: rows of the N dimension are split across 8 NeuronCores (1024
rows each).  Each core receives its a_str row-slice pre-transposed
(a_str[rows].T, bf16) so the kernel works in a [j, i] layout and never
transposes the big matrix on chip, plus replicated h.T / eps.T (fp16)
and small per-core h[rows].T / eps[rows].T used to build this core's
k rows.

Precision plan (validated against the fp32 reference, ~0.5% scale-
relative max error):
  * h/eps/W path is fp16 (scores need ~1e-2 absolute accuracy; fp16
    keeps them there while making every phase-0 matmul single-pass --
    fp32 matmuls cost 2 PE passes on TRN2).
  * scores matmul fp16, accumulated in fp32 PSUM; exp runs on fp32.
  * the a = a_str * exp(scores) tensor and hh_aug are bf16: the final
    matmul is a renormalized weighted average, so per-element rounding
    largely cancels, and bf16 halves the dominant a_str DMA stream.

Algebraic simplifications:
  * softmax's denominator cancels in the renormalization -> skipped.
  * max-subtraction unnecessary: scores ~ N(0, 8^2), exp(|max|~48)
    fits fp32/bf16 range easily.
  * the renorm row-sum falls out of the final matmul by appending a
    ones column to hh (hh_aug, 129 columns per 128-row chunk); one
    per-partition scaled copy performs the division.

PE-array usage:
  * scores have contraction K=EMB=16, so they run 4-way row-packed
    (tile_position=(32r,0)): four j-chunks' score matmuls execute
    concurrently in 32-row strips of the PE array.
  * phase 1 processes groups of (4 j-chunks x 512 i-columns): one
    512KB DMA, 4 packed score matmuls, one [128,2048] exp, one bf16
    multiply, 16 accumulating final matmuls.
"""

import os
import sys

import numpy as np

try:
    import ml_dtypes
    _BF16_NP = ml_dtypes.bfloat16
except ImportError:  # pragma: no cover
    _BF16_NP = None


def _ensure_import_paths():
    for p in (
        "/root/.axon_site",
        "/root/.axon_site/_ro/trn_rl_repo",
        "/root/.axon_site/_ro/pypackages",
        "/opt/trn_rl_repo",
    ):
        if os.path.isdir(p) and p not in sys.path:
            sys.path.append(p)


try:  # noqa: SIM105
    import concourse.bacc  # noqa: F401
except ImportError:
    _ensure_import_paths()

import concourse.bacc as bacc
import concourse.tile as tile
from concourse import mybir
from concourse.bass_utils import run_bass_kernel_spmd

N, IN_F, OUT_F, EMB = 8192, 256, 128, 16
CORES = 8
R = N // CORES            # rows per core (1024)
JC = N // 128             # 64 j-chunks of 128
IC = R // 128             # 8 i-chunks per core
NC_GROUPS = JC // 4       # 16 groups of 4 j-chunks
F32 = mybir.dt.float32
F16 = mybir.dt.float16
BF16 = mybir.dt.bfloat16

_CACHE = {}


def _build():
    """Build + compile the per-core Bass module (identical on all cores)."""
    nc = bacc.Bacc("TRN2", target_bir_lowering=False, debug=False,
                   num_devices=CORES)

    astrT = nc.dram_tensor("astrT", [N, R], BF16, kind="ExternalInput")
    hT = nc.dram_tensor("hT", [IN_F, N], F16, kind="ExternalInput")
    epsT = nc.dram_tensor("epsT", [OUT_F, N], F16, kind="ExternalInput")
    hTc = nc.dram_tensor("hTc", [IN_F, R], F16, kind="ExternalInput")
    epsTc = nc.dram_tensor("epsTc", [OUT_F, R], F16, kind="ExternalInput")
    wfc = nc.dram_tensor("wfc", [IN_F, OUT_F], F16, kind="ExternalInput")
    wk = nc.dram_tensor("wk", [OUT_F, EMB], F16, kind="ExternalInput")
    wq = nc.dram_tensor("wq", [OUT_F, EMB], F16, kind="ExternalInput")
    idn = nc.dram_tensor("idn", [128, 128], F16, kind="ExternalInput")
    outc = nc.dram_tensor("outc", [R, OUT_F], F32, kind="ExternalOutput")

    EXP = mybir.ActivationFunctionType.Exp
    COPY = mybir.ActivationFunctionType.Copy
    MUL = mybir.AluOpType.mult

    with tile.TileContext(nc) as tc:
        with tc.tile_pool(name="persist", bufs=1) as pp:
            # Long-lived SBUF tensors.
            hh_aug = pp.tile([128, JC * 129], BF16, tag="hh_aug")
            qT_pk = pp.tile([128, NC_GROUPS * 128], F16, tag="qT_pk")
            kTc_pk = pp.tile([128, R], F16, tag="kTc_pk")
            wfc0 = pp.tile([128, OUT_F], F16, tag="wfc0")
            wfc1 = pp.tile([128, OUT_F], F16, tag="wfc1")
            wk_sb = pp.tile([OUT_F, EMB], F16, tag="wk")
            wq_sb = pp.tile([OUT_F, EMB], F16, tag="wq")
            idn_sb = pp.tile([128, 128], F16, tag="idn")

            nc.sync.dma_start(out=wfc0[:, :], in_=wfc[0:128, :])
            nc.sync.dma_start(out=wfc1[:, :], in_=wfc[128:256, :])
            nc.sync.dma_start(out=wk_sb[:, :], in_=wk[:, :])
            nc.sync.dma_start(out=wq_sb[:, :], in_=wq[:, :])
            nc.sync.dma_start(out=idn_sb[:, :], in_=idn[:, :])

            # ---------------- Phase 0 ----------------
            with tc.tile_pool(name="ph0", bufs=1) as p0, \
                 tc.tile_pool(name="ph0s", bufs=2) as p0s, \
                 tc.tile_pool(name="ph0ps", bufs=2, space="PSUM") as p0ps, \
                 tc.tile_pool(name="ph0t", bufs=2, space="PSUM") as p0t:
                hhT_sb = p0.tile([OUT_F, N], F16, tag="hhT")
                epsT_sb = p0.tile([OUT_F, N], F16, tag="epsT")
                qT_st = p0.tile([EMB, N], F16, tag="qT_st")
                kTc_sb = p0.tile([EMB, R], F16, tag="kTc_st")
                hT0_sb = p0.tile([128, N], F16, tag="hT0")
                hT1_sb = p0.tile([128, N], F16, tag="hT1")
                ph0_gates = []
                for hs in (slice(0, 2048), slice(2048, 4096),
                           slice(4096, 8192)):
                    d1 = nc.sync.dma_start(out=hT0_sb[:, hs],
                                           in_=hT[0:128, hs])
                    d2 = nc.sync.dma_start(out=hT1_sb[:, hs],
                                           in_=hT[128:256, hs])
                    d3 = nc.gpsimd.dma_start(out=epsT_sb[:, hs],
                                             in_=epsT[:, hs])
                ph0_gates = [d2.ins, d3.ins]

                # This core's k rows first (small, unblocks phase 1):
                # hhTc = W_fc.T @ hTc + epsTc, kTc = W_k.T @ hhTc,
                # replicated to all 4 PE row strips.
                hhTc_sb = p0.tile([OUT_F, R], F16, tag="hhTc")
                htc0 = p0.tile([128, R], F16, tag="htc0")
                htc1 = p0.tile([128, R], F16, tag="htc1")
                epstc = p0.tile([OUT_F, R], F16, tag="epstc")
                nc.gpsimd.dma_start(out=htc0[:, :], in_=hTc[0:128, :])
                nc.gpsimd.dma_start(out=htc1[:, :], in_=hTc[128:256, :])
                nc.gpsimd.dma_start(out=epstc[:, :], in_=epsTc[:, :])
                for f2 in range(2):
                    fs = slice(f2 * 512, (f2 + 1) * 512)
                    psc = p0ps.tile([128, 512], F32, tag="ps_hhT")
                    nc.tensor.matmul(psc[:, :], wfc0[:, :], htc0[:, fs],
                                     start=True, stop=False)
                    nc.tensor.matmul(psc[:, :], wfc1[:, :], htc1[:, fs],
                                     start=False, stop=True)
                    nc.vector.tensor_tensor(hhTc_sb[:, fs], psc[:, :],
                                            epstc[:, fs],
                                            mybir.AluOpType.add)
                for f2 in range(2):
                    fs = slice(f2 * 512, (f2 + 1) * 512)
                    psk = p0ps.tile([EMB, 512], F32, tag="ps_q")
                    nc.tensor.matmul(psk[:, :], wk_sb[:, :], hhTc_sb[:, fs],
                                     start=True, stop=True)
                    nc.vector.tensor_copy(kTc_sb[:, fs], psk[:, :])
                for r in range(4):
                    nc.gpsimd.dma_start(
                        out=kTc_pk[32 * r:32 * r + 16, :], in_=kTc_sb[:, :])

                # hhT = W_fc.T @ hT + epsT, 16 column chunks of 512;
                # qT staging right behind each chunk.
                for f in range(16):
                    fs = slice(f * 512, (f + 1) * 512)
                    ps = p0ps.tile([128, 512], F32, tag="ps_hhT")
                    nc.tensor.matmul(ps[:, :], wfc0[:, :], hT0_sb[:, fs],
                                     start=True, stop=False)
                    nc.tensor.matmul(ps[:, :], wfc1[:, :], hT1_sb[:, fs],
                                     start=False, stop=True)
                    nc.vector.tensor_tensor(hhT_sb[:, fs], ps[:, :],
                                            epsT_sb[:, fs],
                                            mybir.AluOpType.add)
                    psq = p0ps.tile([EMB, 512], F32, tag="ps_q")
                    nc.tensor.matmul(psq[:, :], wq_sb[:, :], hhT_sb[:, fs],
                                     start=True, stop=True)
                    nc.scalar.activation(qT_st[:, fs], psq[:, :],
                                         mybir.ActivationFunctionType.Copy)
                    pst = p0t.tile([128, 512], F16, tag="ps_t", name="pst")
                    for jj in range(4):
                        j = f * 4 + jj
                        js = slice(j * 128, (j + 1) * 128)
                        nc.tensor.matmul(pst[:, jj * 128:(jj + 1) * 128],
                                         hhT_sb[:, js],
                                         idn_sb[:, :], is_transpose=True,
                                         start=True, stop=True)
                    dst = hh_aug[:, :].rearrange(
                        "p (j c) -> p j c", c=129)[:, f * 4:f * 4 + 4, 0:128]
                    src4 = pst[:, :].rearrange("p (j c) -> p j c", c=128)
                    nc.vector.tensor_copy(dst, src4)

                # hh_aug ones columns.
                nc.vector.memset(hh_aug[:, 128::129], 1.0)


                qst3 = qT_st[:, :].rearrange("p (c blk) -> p c blk",
                                             blk=128)
                for r in range(4):
                    dstq = qT_pk[32 * r:32 * r + 16, :].rearrange(
                        "p (c blk) -> p c blk", blk=128)
                    nc.gpsimd.dma_start(out=dstq, in_=qst3[:, r::4, :])

            # ---------------- Phase 1 ----------------
            # Groups of (4 j-chunks x 512 i-columns): 4-way row-packed
            # score matmuls -> exp -> bf16 multiply -> 16 accumulating
            # final matmuls.  8 accumulators of [128,129] pack 3 per
            # PSUM bank; each bank is one accumulation group
            # (per-element has_written handles first-write-overwrites).
            order = [(c, fh, hg, r2, k)
                     for c in range(NC_GROUPS)
                     for fh in range(2)
                     for hg in range(2)
                     for r2 in range(2)
                     for k in range(4)]
            bank_first = {}
            bank_last = {}
            for pos, (c, fh, hg, r2, k) in enumerate(order):
                b = (fh * 4 + k) // 3
                bank_first.setdefault(b, pos)
                bank_last[b] = pos

            with tc.tile_pool(name="acc", bufs=1, space="PSUM") as accp, \
                 tc.tile_pool(name="sps", bufs=2, space="PSUM") as spsp, \
                 tc.tile_pool(name="astr", bufs=8) as astrp, \
                 tc.tile_pool(name="aT", bufs=4) as aTp, \
                 tc.tile_pool(name="outp", bufs=2) as outp:
                accb = [accp.tile([128, 512], F32, tag=f"acc{b}",
                                  name=f"acc{b}")
                        for b in range(3)]

                def acc_ap(i):
                    return accb[i // 3][:, (i % 3) * 129:(i % 3) * 129 + 129]

                pos = 0
                for c in range(NC_GROUPS):
                    for fh in range(2):
                        fhs = slice(fh * 512, (fh + 1) * 512)
                        at = astrp.tile([128, 2048], BF16, tag="astr_t")
                        src = astrT[c * 512:(c + 1) * 512, fhs].rearrange(
                            "(r p) f -> p r f", p=128)
                        dst = at[:, :].rearrange("p (r f) -> p r f", f=512)
                        dd = nc.sync.dma_start(out=dst, in_=src)
                        for g in ph0_gates:
                            tile.add_dep_helper(
                                dd.ins, g, sync=False,
                                reason="astr prefetch after phase-0 inputs")

                        for hg in range(2):
                            pss = spsp.tile([128, 1024], F32, tag="ps_s")
                            for r2 in range(2):
                                r = hg * 2 + r2
                                nc.tensor.matmul(
                                    pss[:, r2 * 512:(r2 + 1) * 512],
                                    qT_pk[32 * r:32 * r + 16,
                                          c * 128:(c + 1) * 128],
                                    kTc_pk[32 * r:32 * r + 16, fhs],
                                    start=True, stop=True,
                                    tile_position=(32 * r, 0))

                            aT_t = aTp.tile([128, 1024], BF16, tag="aT_t")
                            nc.scalar.activation(aT_t[:, :], pss[:, :], EXP)
                            nc.vector.tensor_tensor(
                                aT_t[:, :], aT_t[:, :],
                                at[:, hg * 1024:(hg + 1) * 1024], MUL)

                            for r2 in range(2):
                                r = hg * 2 + r2
                                j = c * 4 + r
                                rhs = hh_aug[:, j * 129:(j + 1) * 129]
                                for k in range(4):
                                    i = fh * 4 + k
                                    b = i // 3
                                    nc.tensor.matmul(
                                        acc_ap(i),
                                        aT_t[:, r2 * 512 + k * 128:
                                             r2 * 512 + (k + 1) * 128],
                                        rhs,
                                        start=(bank_first[b] == pos),
                                        stop=(bank_last[b] == pos),
                                        skip_group_check=True)
                                    pos += 1

                # Epilogue: out rows = numerator / row-sum.
                for i in range(IC):
                    ap = acc_ap(i)
                    rec = outp.tile([128, 1], F32, tag="rec")
                    nc.vector.reciprocal(rec[:, :], ap[:, 128:129])
                    ot = outp.tile([128, OUT_F], F32, tag="ot")
                    nc.vector.tensor_scalar_mul(ot[:, :], ap[:, 0:128],
                                                rec[:, 0:1])
                    eng = nc.scalar if i % 2 == 0 else nc.sync
                    eng.dma_start(
                        out=outc[i * 128:(i + 1) * 128, :], in_=ot[:, :])

    nc.compile()
    return nc


def get_module():
    if "nc" not in _CACHE:
        _CACHE["nc"] = _build()
    return _CACHE["nc"]


def make_in_maps(h, eps, a_str, W_fc, W_k, W_q):
    h = np.asarray(h, np.float32)
    eps = np.asarray(eps, np.float32)
    a_str = np.asarray(a_str, np.float32)
    hT = np.ascontiguousarray(h.T).astype(np.float16)
    epsT = np.ascontiguousarray(eps.T).astype(np.float16)
    idn = np.eye(128, dtype=np.float16)
    common = {
        "hT": hT,
        "epsT": epsT,
        "wfc": np.asarray(W_fc, np.float32).astype(np.float16),
        "wk": np.asarray(W_k, np.float32).astype(np.float16),
        "wq": np.asarray(W_q, np.float32).astype(np.float16),
        "idn": idn,
    }
    in_maps = []
    for c in range(CORES):
        rows = slice(c * R, (c + 1) * R)
        in_maps.append({
            "astrT": np.ascontiguousarray(a_str[rows].T).astype(_BF16_NP),
            "hTc": np.ascontiguousarray(h[rows].T).astype(np.float16),
            "epsTc": np.ascontiguousarray(eps[rows].T).astype(np.float16),
            **common,
        })
    return in_maps


def kernel(h, eps, a_str, W_fc, W_k, W_q):
    nc = get_module()
    in_maps = make_in_maps(h, eps, a_str, W_fc, W_k, W_q)
    res = run_bass_kernel_spmd(nc, in_maps, list(range(CORES)))
    out = np.concatenate([res.results[c]["outc"] for c in range(CORES)],
                         axis=0)
    return out.astype(np.float32)
